# revision 11
# baseline (speedup 1.0000x reference)
"""Trainium2 Bass kernel for DepthwiseXCorr (SiamRPN-style) model.

Pipeline (per sample): conv3x3+BN+ReLU on kernel & search branches,
depthwise cross-correlation, 1x1 conv + BN + ReLU head, 1x1 conv + bias.

Sharding: data-parallel over batch across 8 NeuronCores (8 samples each),
weights replicated.  BN is folded into conv weights on the host.

Layout on device: channels on SBUF partitions (2 chunks of 128), spatial x
batch on the free dimension.  Convolutions run as 9 shifted matmuls (fp32r
for full PE rate), the depthwise xcorr is split across PE (diagonal-matmul
trick), DVE and GPSIMD per (sample, channel-chunk) pair, tuned so every
engine finishes around the same time.
"""

import sys

if "/opt/trn_rl_repo" not in sys.path:
    sys.path.insert(0, "/opt/trn_rl_repo")

from contextlib import ExitStack

import numpy as np

import concourse.bass as bass
import concourse.tile as tile
from concourse import bacc, mybir
from concourse.bass_utils import run_bass_kernel_spmd

EPS = 1e-5
NCORES = 8
B, C, HID, OUT = 64, 256, 256, 10
BPC = B // NCORES  # samples per core
P = 128
KC = C // P  # channel chunks (2)
F32 = mybir.dt.float32
F32R = mybir.dt.float32r
AF = mybir.ActivationFunctionType
OP = mybir.AluOpType

# xcorr engine per (b, cc) pair, index p = b*2 + cc
# 't' = TensorE diag-matmul, 'v' = VectorE, 'g' = GpSimd
XC_ENGINE = ["v", "t", "v", "t", "v", "t", "v", "v",
             "t", "v", "t", "v", "t", "v", "v", "t"]

LAST_RESULTS = None  # BassKernelResults of the most recent run (for profiling)

_prog_cache = {}


def _emit(nc, tc, ctx, d):
    """Emit the per-core program.  d maps dram tensor name -> handle."""
    wp = ctx.enter_context(tc.tile_pool(name="weights", bufs=1))
    srp = ctx.enter_context(tc.tile_pool(name="srelu", bufs=1))
    krp = ctx.enter_context(tc.tile_pool(name="krelu", bufs=1))
    ps_conv = ctx.enter_context(tc.tile_pool(name="ps_conv", bufs=3, space="PSUM"))
    ps_x = ctx.enter_context(tc.tile_pool(name="ps_x", bufs=2, space="PSUM"))
    ps_h1 = ctx.enter_context(tc.tile_pool(name="ps_h1", bufs=2, space="PSUM"))
    ps_h2 = ctx.enter_context(tc.tile_pool(name="ps_h2", bufs=1, space="PSUM"))

    # ---- weights / constants into SBUF ----
    csw_sb, ckw_sb, h1w_sb, h2w_sb = [], [], [], []
    csb_sb, ckb_sb, h1b_sb = [], [], []
    for kc in range(KC):
        t = wp.tile([P, 9 * 2 * P], F32R, tag=f"csw{kc}")
        nc.sync.dma_start(t[:], d["csw"].ap()[kc])
        csw_sb.append(t)
        t = wp.tile([P, 9 * 2 * P], F32R, tag=f"ckw{kc}")
        nc.sync.dma_start(t[:], d["ckw"].ap()[kc])
        ckw_sb.append(t)
        t = wp.tile([P, 2 * P], F32R, tag=f"h1w{kc}")
        nc.sync.dma_start(t[:], d["h1w"].ap()[kc])
        h1w_sb.append(t)
        t = wp.tile([P, OUT], F32R, tag=f"h2w{kc}")
        nc.sync.dma_start(t[:], d["h2w"].ap()[kc])
        h2w_sb.append(t)
    for mc in range(KC):
        t = wp.tile([P, 1], F32, tag=f"csb{mc}")
        nc.sync.dma_start(t[:], d["cs_bias"].ap()[mc])
        csb_sb.append(t)
        t = wp.tile([P, 1], F32, tag=f"ckb{mc}")
        nc.sync.dma_start(t[:], d["ck_bias"].ap()[mc])
        ckb_sb.append(t)
        t = wp.tile([P, 1], F32, tag=f"h1b{mc}")
        nc.sync.dma_start(t[:], d["h1_bias"].ap()[mc])
        h1b_sb.append(t)
    h2b_sb = wp.tile([OUT, 1], F32, tag="h2b")
    nc.sync.dma_start(h2b_sb[:], d["h2_bias"].ap())
    ident_sb = wp.tile([P, P], F32, tag="ident")
    nc.sync.dma_start(ident_sb[:], d["ident"].ap())

    # ---- inputs + convolutions (search pool scoped to this block) ----
    krelu_sb = [krp.tile([P, BPC * 25], F32, tag=f"krelu{mc}", name=f"krelu{mc}") for mc in range(KC)]
    srelu_sb = [srp.tile([P, BPC, 29, 30], F32R, tag=f"srelu{mc}", name=f"srelu{mc}") for mc in range(KC)]
    with tc.tile_pool(name="search", bufs=1) as sp:  # closed after conv_search
        k_sb = []
        for kc in range(KC):
            t = sp.tile([P, BPC, 9, 9], F32R, tag=f"kin{kc}")
            nc.sync.dma_start(t[:], d["k_in"].ap()[kc])
            k_sb.append(t)
        s_sb = [sp.tile([P, BPC, 31, 32], F32R, tag=f"sin{kc}", name=f"sin{kc}") for kc in range(KC)]
        for b in range(BPC):
            for kc in range(KC):
                nc.sync.dma_start(s_sb[kc][:, b], d["s_in"].ap()[kc, :, b])

        # conv_kernel (3x3, BN+ReLU folded): k_relu[mc] = [128, b*25+tap]
        for mc in range(KC):
            psk = ps_conv.tile([P, BPC, 6, 6], F32, tag="pss")
            i = 0
            for tap in range(9):
                dy, dx = tap // 3, tap % 3
                for kc in range(KC):
                    lhsT = ckw_sb[kc][:, tap * 2 * P + mc * P:tap * 2 * P + (mc + 1) * P]
                    rhs = k_sb[kc][:, :, dy:dy + 6, dx:dx + 6]
                    nc.tensor.matmul(psk[:], lhsT, rhs, start=(i == 0), stop=(i == 17))
                    i += 1
            nc.scalar.activation(krelu_sb[mc][:], psk[:, :, 0:5, 0:5], AF.Relu, bias=ckb_sb[mc][:])

        # conv_search: s_relu[mc] = [128, b, 29, 29]
        for b in range(BPC):
            for mc in range(KC):
                for y0, nr in ((0, 15), (15, 14)):
                    pss = ps_conv.tile([P, nr, 30], F32, tag="pss")
                    i = 0
                    for tap in range(9):
                        dy, dx = tap // 3, tap % 3
                        for kc in range(KC):
                            lhsT = csw_sb[kc][:, tap * 2 * P + mc * P:tap * 2 * P + (mc + 1) * P]
                            rhs = s_sb[kc][:, b, y0 + dy:y0 + dy + nr, dx:dx + 30]
                            nc.tensor.matmul(pss[:], lhsT, rhs,
                                             start=(i == 0), stop=(i == 17))
                            i += 1
                    nc.scalar.activation(srelu_sb[mc][:, b, y0:y0 + nr, :], pss[:],
                                         AF.Relu, bias=csb_sb[mc][:])

    # ---- depthwise xcorr + head, pipelined per sample ----
    featp = ctx.enter_context(tc.tile_pool(name="feat", bufs=6))
    diagp = ctx.enter_context(tc.tile_pool(name="diag", bufs=4))
    xrp = ctx.enter_context(tc.tile_pool(name="xrelu", bufs=1))
    outp = ctx.enter_context(tc.tile_pool(name="outs", bufs=1))
    xrelu_sb = [xrp.tile([P, BPC * 625 + 1], F32R, tag=f"xrelu{mc}", name=f"xrelu{mc}") for mc in range(KC)]
    for mc in range(KC):
        nc.vector.memset(xrelu_sb[mc][:, BPC * 625:].bitcast(F32), 0.0)
    out_sb = outp.tile([OUT, BPC * 625], F32, tag="osb")

    def kscalar(cc, b, tap):
        return krelu_sb[cc][:, b * 25 + tap:b * 25 + tap + 1]

    def win(cc, b, tap, r0=0, nr=25, w=25):
        dy, dx = tap // 5, tap % 5
        return srelu_sb[cc][:, b, dy + r0:dy + r0 + nr, dx:dx + w]

    for b in range(BPC):
        feat = []
        for cc in range(KC):
            eng = XC_ENGINE[b * 2 + cc]
            ft = featp.tile([P, 626], F32R, tag="feat")
            nc.vector.memset(ft[:, 625:626].bitcast(F32), 0.0)
            if eng in ("v", "g"):
                e = nc.vector if eng == "v" else nc.gpsimd
                e.tensor_scalar(ft[:, 0:625], win(cc, b, 0).bitcast(F32), kscalar(cc, b, 0), None, OP.mult)
                for tap in range(1, 25):
                    e.scalar_tensor_tensor(ft[:, 0:625], win(cc, b, tap).bitcast(F32), kscalar(cc, b, tap),
                                           ft[:, 0:625].bitcast(F32), OP.mult, OP.add)
            else:  # TensorE: accumulate diag(k_tap) @ shifted windows in PSUM
                ps_a = ps_x.tile([P, 13, 26], F32, tag="psx")
                ps_b = ps_x.tile([P, 12, 26], F32, tag="psx")
                for tap in range(25):
                    dg = diagp.tile([P, P], F32R, tag="diag")
                    nc.gpsimd.tensor_scalar(dg[:], ident_sb[:], kscalar(cc, b, tap),
                                            None, OP.mult)
                    nc.tensor.matmul(ps_a[:], dg[:],
                                     win(cc, b, tap, 0, 13, 26),
                                     start=(tap == 0), stop=(tap == 24))
                    nc.tensor.matmul(ps_b[:], dg[:],
                                     win(cc, b, tap, 13, 12, 26),
                                     start=(tap == 0), stop=(tap == 24))
                nc.scalar.activation(ft[:, 0:325], ps_a[:, :, 0:25], AF.Copy)
                nc.scalar.activation(ft[:, 325:625], ps_b[:, :, 0:25], AF.Copy)
            feat.append(ft)

        # head1: 1x1 conv + BN + ReLU
        for mc in range(KC):
            for o0, n, nv in ((0, 320, 320), (320, 306, 305)):
                ph = ps_h1.tile([P, n], F32, tag="psh1")
                for kc in range(KC):
                    lhsT = h1w_sb[kc][:, mc * P:(mc + 1) * P]
                    nc.tensor.matmul(ph[:], lhsT,
                                     feat[kc][:, o0:o0 + n],
                                     start=(kc == 0), stop=(kc == 1))
                nc.scalar.activation(xrelu_sb[mc][:, b * 625 + o0:b * 625 + o0 + nv],
                                     ph[:, 0:nv], AF.Relu, bias=h1b_sb[mc][:])

        # head2: 1x1 conv + bias
        for o0, n, nv in ((0, 320, 320), (320, 306, 305)):
            po = ps_h2.tile([OUT, n], F32, tag="psh2")
            for kc in range(KC):
                nc.tensor.matmul(po[:], h2w_sb[kc][:],
                                 xrelu_sb[kc][:, b * 625 + o0:b * 625 + o0 + n],
                                 start=(kc == 0), stop=(kc == 1))
            nc.scalar.activation(out_sb[:, b * 625 + o0:b * 625 + o0 + nv], po[:, 0:nv],
                                 AF.Identity, bias=h2b_sb[:])

    nc.sync.dma_start(d["out"].ap(), out_sb[:])


def _build_program():
    if "nc" in _prog_cache:
        return _prog_cache["nc"]
    nc = bacc.Bacc("TRN2", target_bir_lowering=False, debug=False,
                   num_devices=NCORES)
    d = {}
    d["s_in"] = nc.dram_tensor("s_in", [KC, P, BPC, 31, 32], F32R, kind="ExternalInput")
    d["k_in"] = nc.dram_tensor("k_in", [KC, P, BPC, 9, 9], F32R, kind="ExternalInput")
    d["csw"] = nc.dram_tensor("csw", [KC, P, 9, 2, P], F32R, kind="ExternalInput")
    d["ckw"] = nc.dram_tensor("ckw", [KC, P, 9, 2, P], F32R, kind="ExternalInput")
    d["cs_bias"] = nc.dram_tensor("cs_bias", [KC, P, 1], F32, kind="ExternalInput")
    d["ck_bias"] = nc.dram_tensor("ck_bias", [KC, P, 1], F32, kind="ExternalInput")
    d["h1w"] = nc.dram_tensor("h1w", [KC, P, 2, P], F32R, kind="ExternalInput")
    d["h1_bias"] = nc.dram_tensor("h1_bias", [KC, P, 1], F32, kind="ExternalInput")
    d["h2w"] = nc.dram_tensor("h2w", [KC, P, OUT], F32R, kind="ExternalInput")
    d["h2_bias"] = nc.dram_tensor("h2_bias", [OUT, 1], F32, kind="ExternalInput")
    d["ident"] = nc.dram_tensor("ident", [P, P], F32, kind="ExternalInput")
    d["out"] = nc.dram_tensor("out", [OUT, BPC * 625], F32, kind="ExternalOutput")

    with tile.TileContext(nc) as tc:
        with ExitStack() as ctx:
            _emit(nc, tc, ctx, d)
    nc.compile()
    _prog_cache["nc"] = nc
    return nc


def kernel(**inputs):
    global LAST_RESULTS
    f32 = lambda x: np.ascontiguousarray(np.asarray(x), dtype=np.float32)
    kern, search = f32(inputs["kernel"]), f32(inputs["search"])

    # fold BN into conv weights / biases
    cks = f32(inputs["ck_g"]) / np.sqrt(f32(inputs["ck_v"]) + EPS)
    ckw_f = f32(inputs["ck_w"]) * cks[:, None, None, None]
    ckb = f32(inputs["ck_b"]) - f32(inputs["ck_m"]) * cks
    css = f32(inputs["cs_g"]) / np.sqrt(f32(inputs["cs_v"]) + EPS)
    csw_f = f32(inputs["cs_w"]) * css[:, None, None, None]
    csb = f32(inputs["cs_b"]) - f32(inputs["cs_m"]) * css
    h1s = f32(inputs["h_g"]) / np.sqrt(f32(inputs["h_v"]) + EPS)
    h1w_f = f32(inputs["h1_w"]) * h1s[:, None]
    h1b = f32(inputs["h_b"]) - f32(inputs["h_m"]) * h1s

    shared = {
        "csw": np.ascontiguousarray(csw_f.transpose(1, 2, 3, 0).reshape(KC, P, 9, 2, P)),
        "ckw": np.ascontiguousarray(ckw_f.transpose(1, 2, 3, 0).reshape(KC, P, 9, 2, P)),
        "cs_bias": csb.reshape(KC, P, 1),
        "ck_bias": ckb.reshape(KC, P, 1),
        "h1w": np.ascontiguousarray(h1w_f.transpose(1, 0).reshape(KC, P, 2, P)),
        "h1_bias": h1b.reshape(KC, P, 1),
        "h2w": np.ascontiguousarray(f32(inputs["h2_w"]).transpose(1, 0).reshape(KC, P, OUT)),
        "h2_bias": f32(inputs["h2_b"]).reshape(OUT, 1),
        "ident": np.eye(P, dtype=np.float32),
    }
    in_maps = []
    for i in range(NCORES):
        sl = slice(i * BPC, (i + 1) * BPC)
        m = dict(shared)
        s_pad = np.zeros((KC, P, BPC, 31, 32), np.float32)
        s_pad[..., :31] = search[sl].transpose(1, 0, 2, 3).reshape(KC, P, BPC, 31, 31)
        m["s_in"] = s_pad
        k_pad = np.zeros((KC, P, BPC, 9, 9), np.float32)
        k_pad[..., :7, :7] = kern[sl].transpose(1, 0, 2, 3).reshape(KC, P, BPC, 7, 7)
        m["k_in"] = k_pad
        in_maps.append(m)

    nc = _build_program()
    res = run_bass_kernel_spmd(nc, in_maps, core_ids=list(range(NCORES)))
    LAST_RESULTS = res
    out = np.empty((B, OUT, 25, 25), dtype=np.float32)
    for i in range(NCORES):
        o = res.results[i]["out"].reshape(OUT, BPC, 25, 25)
        out[i * BPC:(i + 1) * BPC] = o.transpose(1, 0, 2, 3)
    return out


# revision 12
# speedup vs baseline: 1.6596x; 1.6596x over previous
"""Trainium2 Bass kernel for DepthwiseXCorr (SiamRPN-style) model.

Pipeline (per sample): conv3x3+BN+ReLU on kernel & search branches,
depthwise cross-correlation, 1x1 conv + BN + ReLU head, 1x1 conv + bias.

Sharding: data-parallel over batch across 8 NeuronCores (8 samples each),
weights replicated.  BN is folded into conv weights on the host.

Layout on device: channels on SBUF partitions (2 chunks of 128), spatial x
batch on the free dimension.  Convolutions run as 9 shifted matmuls (fp32r
for full PE rate), the depthwise xcorr is split across PE (diagonal-matmul
trick), DVE and GPSIMD per (sample, channel-chunk) pair, tuned so every
engine finishes around the same time.
"""

import sys

if "/opt/trn_rl_repo" not in sys.path:
    sys.path.insert(0, "/opt/trn_rl_repo")

from contextlib import ExitStack

import numpy as np

import concourse.bass as bass
import concourse.tile as tile
from concourse import bacc, mybir
from concourse.bass_utils import run_bass_kernel_spmd

EPS = 1e-5
NCORES = 8
B, C, HID, OUT = 64, 256, 256, 10
BPC = B // NCORES  # samples per core
P = 128
KC = C // P  # channel chunks (2)
F32 = mybir.dt.float32
F32R = mybir.dt.float32r
AF = mybir.ActivationFunctionType
OP = mybir.AluOpType

# xcorr engine per (b, cc) pair, index p = b*2 + cc
# 't' = TensorE diag-matmul, 'v' = VectorE, 'g' = GpSimd
XC_ENGINE = ["v", "t", "v", "t", "v", "t", "v", "t",
             "v", "t", "v", "t", "v", "t", "v", "t"]

LAST_RESULTS = None  # BassKernelResults of the most recent run (for profiling)

_prog_cache = {}


def _emit(nc, tc, ctx, d):
    """Emit the per-core program.  d maps dram tensor name -> handle."""
    wp = ctx.enter_context(tc.tile_pool(name="weights", bufs=1))
    srp = ctx.enter_context(tc.tile_pool(name="srelu", bufs=1))
    krp = ctx.enter_context(tc.tile_pool(name="krelu", bufs=1))
    ps_conv = ctx.enter_context(tc.tile_pool(name="ps_conv", bufs=3, space="PSUM"))
    ps_x = ctx.enter_context(tc.tile_pool(name="ps_x", bufs=2, space="PSUM"))
    ps_hd = ctx.enter_context(tc.tile_pool(name="ps_hd", bufs=3, space="PSUM"))

    # ---- weights / constants into SBUF ----
    csw_sb, ckw_sb, h1w_sb, h2w_sb = [], [], [], []
    csb_sb, ckb_sb, h1b_sb = [], [], []
    for kc in range(KC):
        t = wp.tile([P, 9 * 2 * P], F32R, tag=f"csw{kc}")
        nc.sync.dma_start(t[:], d["csw"].ap()[kc])
        csw_sb.append(t)
        t = wp.tile([P, 9 * 2 * P], F32R, tag=f"ckw{kc}")
        nc.sync.dma_start(t[:], d["ckw"].ap()[kc])
        ckw_sb.append(t)
        t = wp.tile([P, 2 * P], F32R, tag=f"h1w{kc}")
        nc.sync.dma_start(t[:], d["h1w"].ap()[kc])
        h1w_sb.append(t)
        t = wp.tile([P, OUT], F32R, tag=f"h2w{kc}")
        nc.sync.dma_start(t[:], d["h2w"].ap()[kc])
        h2w_sb.append(t)
    for mc in range(KC):
        t = wp.tile([P, 1], F32, tag=f"csb{mc}")
        nc.sync.dma_start(t[:], d["cs_bias"].ap()[mc])
        csb_sb.append(t)
        t = wp.tile([P, 1], F32, tag=f"ckb{mc}")
        nc.sync.dma_start(t[:], d["ck_bias"].ap()[mc])
        ckb_sb.append(t)
        t = wp.tile([P, 1], F32, tag=f"h1b{mc}")
        nc.sync.dma_start(t[:], d["h1_bias"].ap()[mc])
        h1b_sb.append(t)
    h2b_sb = wp.tile([OUT, 1], F32, tag="h2b")
    nc.sync.dma_start(h2b_sb[:], d["h2_bias"].ap())
    ident_sb = wp.tile([P, P], F32, tag="ident")
    nc.sync.dma_start(ident_sb[:], d["ident"].ap())

    # ---- inputs + convolutions (search pool scoped to this block) ----
    krelu_sb = [krp.tile([P, BPC * 25], F32, tag=f"krelu{mc}", name=f"krelu{mc}") for mc in range(KC)]
    srelu_sb = [srp.tile([P, BPC, 29, 30], F32R, tag=f"srelu{mc}", name=f"srelu{mc}") for mc in range(KC)]
    with tc.tile_pool(name="search", bufs=1) as sp:  # closed after conv_search
        k_sb = []
        for kc in range(KC):
            t = sp.tile([P, BPC, 9, 9], F32R, tag=f"kin{kc}")
            nc.sync.dma_start(t[:], d["k_in"].ap()[kc])
            k_sb.append(t)
        s_sb = [sp.tile([P, BPC, 31, 32], F32R, tag=f"sin{kc}", name=f"sin{kc}") for kc in range(KC)]
        for b in range(BPC):
            for kc in range(KC):
                nc.sync.dma_start(s_sb[kc][:, b], d["s_in"].ap()[kc, :, b])

        # conv_kernel (3x3, BN+ReLU folded): k_relu[mc] = [128, b*25+tap]
        for mc in range(KC):
            psk = ps_conv.tile([P, BPC, 6, 6], F32, tag="pss")
            i = 0
            for tap in range(9):
                dy, dx = tap // 3, tap % 3
                for kc in range(KC):
                    lhsT = ckw_sb[kc][:, tap * 2 * P + mc * P:tap * 2 * P + (mc + 1) * P]
                    rhs = k_sb[kc][:, :, dy:dy + 6, dx:dx + 6]
                    nc.tensor.matmul(psk[:], lhsT, rhs, start=(i == 0), stop=(i == 17))
                    i += 1
            nc.scalar.activation(krelu_sb[mc][:], psk[:, :, 0:5, 0:5], AF.Relu, bias=ckb_sb[mc][:])

        # conv_search: s_relu[mc] = [128, b, 29, 29]
        for b in range(BPC):
            for mc in range(KC):
                for y0, nr in ((0, 15), (15, 14)):
                    pss = ps_conv.tile([P, nr, 30], F32, tag="pss")
                    i = 0
                    for tap in range(9):
                        dy, dx = tap // 3, tap % 3
                        for kc in range(KC):
                            lhsT = csw_sb[kc][:, tap * 2 * P + mc * P:tap * 2 * P + (mc + 1) * P]
                            rhs = s_sb[kc][:, b, y0 + dy:y0 + dy + nr, dx:dx + 30]
                            nc.tensor.matmul(pss[:], lhsT, rhs,
                                             start=(i == 0), stop=(i == 17))
                            i += 1
                    nc.scalar.activation(srelu_sb[mc][:, b, y0:y0 + nr, :], pss[:],
                                         AF.Relu, bias=csb_sb[mc][:])

    # ---- depthwise xcorr + head, pipelined per sample ----
    featp = ctx.enter_context(tc.tile_pool(name="feat", bufs=6))
    diagp = ctx.enter_context(tc.tile_pool(name="diag", bufs=4))
    xrp = ctx.enter_context(tc.tile_pool(name="xrelu", bufs=1))
    outp = ctx.enter_context(tc.tile_pool(name="outs", bufs=1))
    xrelu_sb = [xrp.tile([P, BPC * 625 + 1], F32R, tag=f"xrelu{mc}", name=f"xrelu{mc}") for mc in range(KC)]
    for mc in range(KC):
        nc.vector.memset(xrelu_sb[mc][:, BPC * 625:].bitcast(F32), 0.0)
    out_sb = outp.tile([OUT, BPC * 625], F32, tag="osb")

    def kscalar(cc, b, tap):
        return krelu_sb[cc][:, b * 25 + tap:b * 25 + tap + 1]

    def win(cc, b, tap, r0=0, nr=25, w=25):
        dy, dx = tap // 5, tap % 5
        return srelu_sb[cc][:, b, dy + r0:dy + r0 + nr, dx:dx + w]

    def win746(cc, b, tap):
        # contiguous 746-elem span of the (dy,dx)-shifted window
        dy, dx = tap // 5, tap % 5
        flat = srelu_sb[cc][:].rearrange("p b y x -> p (b y x)")
        off = b * 870 + dy * 30 + dx
        return flat[:, off:off + 746]

    def ftwin(ft, r0, nr):
        # [nr, 26] row window of the 30-stride feat tile
        return ft[:].rearrange("p (y x) -> p y x", x=30)[:, r0:r0 + nr, 0:26]

    for b in range(BPC):
        feat = []
        for cc in range(KC):
            eng = XC_ENGINE[b * 2 + cc]
            # ft holds the 25x25 xcorr output on a 30-element row stride
            # (cols 25..29 junk) so DVE ops can run one contiguous span.
            ft = featp.tile([P, 750], F32R, tag="feat")
            if eng == "v":
                e = nc.vector
                e.tensor_scalar(ft[:, 0:746], win746(cc, b, 0).bitcast(F32),
                                kscalar(cc, b, 0), None, OP.mult)
                for tap in range(1, 25):
                    e.scalar_tensor_tensor(ft[:, 0:746], win746(cc, b, tap).bitcast(F32),
                                           kscalar(cc, b, tap),
                                           ft[:, 0:746].bitcast(F32), OP.mult, OP.add)
            else:  # TensorE: accumulate diag(k_tap) @ shifted windows in PSUM
                ps_a = ps_x.tile([P, 13, 26], F32, tag="psx")
                ps_b = ps_x.tile([P, 12, 26], F32, tag="psx")
                for tap in range(25):
                    dg = diagp.tile([P, P], F32R, tag="diag")
                    nc.scalar.activation(dg[:], ident_sb[:], AF.Copy,
                                         scale=kscalar(cc, b, tap))
                    nc.tensor.matmul(ps_a[:], dg[:],
                                     win(cc, b, tap, 0, 13, 26),
                                     start=(tap == 0), stop=(tap == 24))
                    nc.tensor.matmul(ps_b[:], dg[:],
                                     win(cc, b, tap, 13, 12, 26),
                                     start=(tap == 0), stop=(tap == 24))
                nc.scalar.activation(ftwin(ft, 0, 13), ps_a[:], AF.Copy)
                nc.scalar.activation(ftwin(ft, 13, 12), ps_b[:], AF.Copy)
            feat.append(ft)

        # head1: 1x1 conv + BN + ReLU (row-aligned splits over the 25x26 window)
        for mc in range(KC):
            for r0, nr in ((0, 13), (13, 12)):
                ph = ps_hd.tile([P, nr, 26], F32, tag="pshd")
                for kc in range(KC):
                    lhsT = h1w_sb[kc][:, mc * P:(mc + 1) * P]
                    nc.tensor.matmul(ph[:], lhsT, ftwin(feat[kc], r0, nr),
                                     start=(kc == 0), stop=(kc == 1))
                nc.scalar.activation(
                    xrelu_sb[mc][:, b * 625 + r0 * 25:b * 625 + (r0 + nr) * 25],
                    ph[:, :, 0:25], AF.Relu, bias=h1b_sb[mc][:])

        # head2: 1x1 conv + bias
        for o0, n, nv in ((0, 320, 320), (320, 306, 305)):
            po = ps_hd.tile([OUT, n], F32, tag="pshd")
            for kc in range(KC):
                nc.tensor.matmul(po[:], h2w_sb[kc][:],
                                 xrelu_sb[kc][:, b * 625 + o0:b * 625 + o0 + n],
                                 start=(kc == 0), stop=(kc == 1))
            nc.scalar.activation(out_sb[:, b * 625 + o0:b * 625 + o0 + nv], po[:, 0:nv],
                                 AF.Identity, bias=h2b_sb[:])

    nc.sync.dma_start(d["out"].ap(), out_sb[:])


def _build_program():
    if "nc" in _prog_cache:
        return _prog_cache["nc"]
    nc = bacc.Bacc("TRN2", target_bir_lowering=False, debug=False,
                   num_devices=NCORES)
    d = {}
    d["s_in"] = nc.dram_tensor("s_in", [KC, P, BPC, 31, 32], F32R, kind="ExternalInput")
    d["k_in"] = nc.dram_tensor("k_in", [KC, P, BPC, 9, 9], F32R, kind="ExternalInput")
    d["csw"] = nc.dram_tensor("csw", [KC, P, 9, 2, P], F32R, kind="ExternalInput")
    d["ckw"] = nc.dram_tensor("ckw", [KC, P, 9, 2, P], F32R, kind="ExternalInput")
    d["cs_bias"] = nc.dram_tensor("cs_bias", [KC, P, 1], F32, kind="ExternalInput")
    d["ck_bias"] = nc.dram_tensor("ck_bias", [KC, P, 1], F32, kind="ExternalInput")
    d["h1w"] = nc.dram_tensor("h1w", [KC, P, 2, P], F32R, kind="ExternalInput")
    d["h1_bias"] = nc.dram_tensor("h1_bias", [KC, P, 1], F32, kind="ExternalInput")
    d["h2w"] = nc.dram_tensor("h2w", [KC, P, OUT], F32R, kind="ExternalInput")
    d["h2_bias"] = nc.dram_tensor("h2_bias", [OUT, 1], F32, kind="ExternalInput")
    d["ident"] = nc.dram_tensor("ident", [P, P], F32, kind="ExternalInput")
    d["out"] = nc.dram_tensor("out", [OUT, BPC * 625], F32, kind="ExternalOutput")

    with tile.TileContext(nc) as tc:
        with ExitStack() as ctx:
            _emit(nc, tc, ctx, d)
    nc.compile()
    _prog_cache["nc"] = nc
    return nc


def kernel(**inputs):
    global LAST_RESULTS
    f32 = lambda x: np.ascontiguousarray(np.asarray(x), dtype=np.float32)
    kern, search = f32(inputs["kernel"]), f32(inputs["search"])

    # fold BN into conv weights / biases
    cks = f32(inputs["ck_g"]) / np.sqrt(f32(inputs["ck_v"]) + EPS)
    ckw_f = f32(inputs["ck_w"]) * cks[:, None, None, None]
    ckb = f32(inputs["ck_b"]) - f32(inputs["ck_m"]) * cks
    css = f32(inputs["cs_g"]) / np.sqrt(f32(inputs["cs_v"]) + EPS)
    csw_f = f32(inputs["cs_w"]) * css[:, None, None, None]
    csb = f32(inputs["cs_b"]) - f32(inputs["cs_m"]) * css
    h1s = f32(inputs["h_g"]) / np.sqrt(f32(inputs["h_v"]) + EPS)
    h1w_f = f32(inputs["h1_w"]) * h1s[:, None]
    h1b = f32(inputs["h_b"]) - f32(inputs["h_m"]) * h1s

    shared = {
        "csw": np.ascontiguousarray(csw_f.transpose(1, 2, 3, 0).reshape(KC, P, 9, 2, P)),
        "ckw": np.ascontiguousarray(ckw_f.transpose(1, 2, 3, 0).reshape(KC, P, 9, 2, P)),
        "cs_bias": csb.reshape(KC, P, 1),
        "ck_bias": ckb.reshape(KC, P, 1),
        "h1w": np.ascontiguousarray(h1w_f.transpose(1, 0).reshape(KC, P, 2, P)),
        "h1_bias": h1b.reshape(KC, P, 1),
        "h2w": np.ascontiguousarray(f32(inputs["h2_w"]).transpose(1, 0).reshape(KC, P, OUT)),
        "h2_bias": f32(inputs["h2_b"]).reshape(OUT, 1),
        "ident": np.eye(P, dtype=np.float32),
    }
    in_maps = []
    for i in range(NCORES):
        sl = slice(i * BPC, (i + 1) * BPC)
        m = dict(shared)
        s_pad = np.zeros((KC, P, BPC, 31, 32), np.float32)
        s_pad[..., :31] = search[sl].transpose(1, 0, 2, 3).reshape(KC, P, BPC, 31, 31)
        m["s_in"] = s_pad
        k_pad = np.zeros((KC, P, BPC, 9, 9), np.float32)
        k_pad[..., :7, :7] = kern[sl].transpose(1, 0, 2, 3).reshape(KC, P, BPC, 7, 7)
        m["k_in"] = k_pad
        in_maps.append(m)

    nc = _build_program()
    res = run_bass_kernel_spmd(nc, in_maps, core_ids=list(range(NCORES)))
    LAST_RESULTS = res
    out = np.empty((B, OUT, 25, 25), dtype=np.float32)
    for i in range(NCORES):
        o = res.results[i]["out"].reshape(OUT, BPC, 25, 25)
        out[i * BPC:(i + 1) * BPC] = o.transpose(1, 0, 2, 3)
    return out


# revision 14
# speedup vs baseline: 1.7121x; 1.0316x over previous
"""Trainium2 Bass kernel for DepthwiseXCorr (SiamRPN-style) model.

Pipeline (per sample): conv3x3+BN+ReLU on kernel & search branches,
depthwise cross-correlation, 1x1 conv + BN + ReLU head, 1x1 conv + bias.

Sharding: data-parallel over batch across 8 NeuronCores (8 samples each),
weights replicated.  BN is folded into conv weights on the host.

Layout on device: channels on SBUF partitions (2 chunks of 128), spatial x
batch on the free dimension.  Convolutions run as 9 shifted matmuls (fp32r
for full PE rate), the depthwise xcorr is split across PE (diagonal-matmul
trick), DVE and GPSIMD per (sample, channel-chunk) pair, tuned so every
engine finishes around the same time.
"""

import sys

if "/opt/trn_rl_repo" not in sys.path:
    sys.path.insert(0, "/opt/trn_rl_repo")

from contextlib import ExitStack

import numpy as np

import concourse.bass as bass
import concourse.tile as tile
from concourse import bacc, mybir
from concourse.bass_utils import run_bass_kernel_spmd

EPS = 1e-5
NCORES = 8
B, C, HID, OUT = 64, 256, 256, 10
BPC = B // NCORES  # samples per core
P = 128
KC = C // P  # channel chunks (2)
F32 = mybir.dt.float32
F32R = mybir.dt.float32r
AF = mybir.ActivationFunctionType
OP = mybir.AluOpType

# xcorr engine per (b, cc) pair, index p = b*2 + cc
# 't' = TensorE diag-matmul, 'v' = VectorE, 'g' = GpSimd
XC_ENGINE = ["v", "t", "v", "t", "v", "t", "v", "t",
             "v", "t", "v", "t", "v", "t", "v", "t"]

LAST_RESULTS = None  # BassKernelResults of the most recent run (for profiling)

_prog_cache = {}


def _emit(nc, tc, ctx, d):
    """Emit the per-core program.  d maps dram tensor name -> handle."""
    wp = ctx.enter_context(tc.tile_pool(name="weights", bufs=1))
    srp = ctx.enter_context(tc.tile_pool(name="srelu", bufs=1))
    krp = ctx.enter_context(tc.tile_pool(name="krelu", bufs=1))
    kp = ctx.enter_context(tc.tile_pool(name="kern", bufs=1))
    sp = ctx.enter_context(tc.tile_pool(name="search", bufs=6))
    featp = ctx.enter_context(tc.tile_pool(name="feat", bufs=6))
    diagp = ctx.enter_context(tc.tile_pool(name="diag", bufs=4))
    xrp = ctx.enter_context(tc.tile_pool(name="xrelu", bufs=6))
    outp = ctx.enter_context(tc.tile_pool(name="outs", bufs=1))
    ps_conv = ctx.enter_context(tc.tile_pool(name="ps_conv", bufs=3, space="PSUM"))
    ps_x = ctx.enter_context(tc.tile_pool(name="ps_x", bufs=2, space="PSUM"))
    ps_hd = ctx.enter_context(tc.tile_pool(name="ps_hd", bufs=3, space="PSUM"))

    # ---- weights / constants into SBUF ----
    csw_sb, ckw_sb, h1w_sb, h2w_sb = [], [], [], []
    csb_sb, ckb_sb, h1b_sb = [], [], []
    for kc in range(KC):
        t = wp.tile([P, 9 * 2 * P], F32R, tag=f"csw{kc}")
        nc.sync.dma_start(t[:], d["csw"].ap()[kc])
        csw_sb.append(t)
        t = wp.tile([P, 9 * 2 * P], F32R, tag=f"ckw{kc}")
        nc.sync.dma_start(t[:], d["ckw"].ap()[kc])
        ckw_sb.append(t)
        t = wp.tile([P, 2 * P], F32R, tag=f"h1w{kc}")
        nc.sync.dma_start(t[:], d["h1w"].ap()[kc])
        h1w_sb.append(t)
        t = wp.tile([P, OUT], F32R, tag=f"h2w{kc}")
        nc.sync.dma_start(t[:], d["h2w"].ap()[kc])
        h2w_sb.append(t)
    for mc in range(KC):
        t = wp.tile([P, 1], F32, tag=f"csb{mc}")
        nc.sync.dma_start(t[:], d["cs_bias"].ap()[mc])
        csb_sb.append(t)
        t = wp.tile([P, 1], F32, tag=f"ckb{mc}")
        nc.sync.dma_start(t[:], d["ck_bias"].ap()[mc])
        ckb_sb.append(t)
        t = wp.tile([P, 1], F32, tag=f"h1b{mc}")
        nc.sync.dma_start(t[:], d["h1_bias"].ap()[mc])
        h1b_sb.append(t)
    h2b_sb = wp.tile([OUT, 1], F32, tag="h2b")
    nc.sync.dma_start(h2b_sb[:], d["h2_bias"].ap())
    ident_sb = wp.tile([P, P], F32, tag="ident")
    nc.sync.dma_start(ident_sb[:], d["ident"].ap())

    # ---- inputs + convolutions (rotating per-sample search tiles) ----
    krelu_sb = [krp.tile([P, BPC * 25], F32, tag=f"krelu{mc}", name=f"krelu{mc}") for mc in range(KC)]
    srelu_sb = [srp.tile([P, BPC, 29, 30], F32R, tag=f"srelu{mc}", name=f"srelu{mc}") for mc in range(KC)]
    k_sb = []
    for kc in range(KC):
        t = kp.tile([P, BPC, 9, 9], F32R, tag=f"kin{kc}")
        nc.sync.dma_start(t[:], d["k_in"].ap()[kc])
        k_sb.append(t)

    # conv_kernel (3x3, BN+ReLU folded): k_relu[mc] = [128, b*25+tap]
    for mc in range(KC):
        psk = ps_conv.tile([P, BPC, 6, 6], F32, tag="pss")
        i = 0
        for tap in range(9):
            dy, dx = tap // 3, tap % 3
            for kc in range(KC):
                lhsT = ckw_sb[kc][:, tap * 2 * P + mc * P:tap * 2 * P + (mc + 1) * P]
                rhs = k_sb[kc][:, :, dy:dy + 6, dx:dx + 6]
                nc.tensor.matmul(psk[:], lhsT, rhs, start=(i == 0), stop=(i == 17))
                i += 1
        nc.scalar.activation(krelu_sb[mc][:], psk[:, :, 0:5, 0:5], AF.Relu, bias=ckb_sb[mc][:])

    # conv_search: s_relu[mc] = [128, b, 29, 30] (col 29 = padding junk)
    for b in range(BPC):
        s_sb = []
        for kc in range(KC):
            t = sp.tile([P, 31, 32], F32R, tag="sin", name=f"sin{kc}_{b}")
            nc.sync.dma_start(t[:], d["s_in"].ap()[kc, :, b])
            s_sb.append(t)
        for mc in range(KC):
            for y0, nr in ((0, 15), (15, 14)):
                pss = ps_conv.tile([P, nr, 30], F32, tag="pss")
                i = 0
                for tap in range(9):
                    dy, dx = tap // 3, tap % 3
                    for kc in range(KC):
                        lhsT = csw_sb[kc][:, tap * 2 * P + mc * P:tap * 2 * P + (mc + 1) * P]
                        rhs = s_sb[kc][:, y0 + dy:y0 + dy + nr, dx:dx + 30]
                        nc.tensor.matmul(pss[:], lhsT, rhs,
                                         start=(i == 0), stop=(i == 17))
                        i += 1
                nc.scalar.activation(srelu_sb[mc][:, b, y0:y0 + nr, :], pss[:],
                                     AF.Relu, bias=csb_sb[mc][:])

    # ---- depthwise xcorr + head, pipelined per sample ----
    out_sb = outp.tile([OUT, BPC * 625], F32, tag="osb")

    def kscalar(cc, b, tap):
        return krelu_sb[cc][:, b * 25 + tap:b * 25 + tap + 1]

    def win(cc, b, tap, r0=0, nr=25, w=25):
        dy, dx = tap // 5, tap % 5
        return srelu_sb[cc][:, b, dy + r0:dy + r0 + nr, dx:dx + w]

    def win746(cc, b, tap):
        # contiguous 746-elem span of the (dy,dx)-shifted window
        dy, dx = tap // 5, tap % 5
        flat = srelu_sb[cc][:].rearrange("p b y x -> p (b y x)")
        off = b * 870 + dy * 30 + dx
        return flat[:, off:off + 746]

    def ftwin(ft, r0, nr):
        # [nr, 26] row window of the 30-stride feat tile
        return ft[:].rearrange("p (y x) -> p y x", x=30)[:, r0:r0 + nr, 0:26]

    for b in range(BPC):
        feat = []
        for cc in range(KC):
            eng = XC_ENGINE[b * 2 + cc]
            # ft holds the 25x25 xcorr output on a 30-element row stride
            # (cols 25..29 junk) so DVE ops can run one contiguous span.
            ft = featp.tile([P, 750], F32R, tag="feat")
            if eng == "v":
                e = nc.vector
                e.tensor_scalar(ft[:, 0:746], win746(cc, b, 0).bitcast(F32),
                                kscalar(cc, b, 0), None, OP.mult)
                for tap in range(1, 25):
                    e.scalar_tensor_tensor(ft[:, 0:746], win746(cc, b, tap).bitcast(F32),
                                           kscalar(cc, b, tap),
                                           ft[:, 0:746].bitcast(F32), OP.mult, OP.add)
            else:  # TensorE: accumulate diag(k_tap) @ shifted windows in PSUM
                ps_a = ps_x.tile([P, 13, 26], F32, tag="psx")
                ps_b = ps_x.tile([P, 12, 26], F32, tag="psx")
                for tap in range(25):
                    dg = diagp.tile([P, P], F32R, tag="diag")
                    nc.scalar.activation(dg[:], ident_sb[:], AF.Copy,
                                         scale=kscalar(cc, b, tap))
                    nc.tensor.matmul(ps_a[:], dg[:],
                                     win(cc, b, tap, 0, 13, 26),
                                     start=(tap == 0), stop=(tap == 24))
                    nc.tensor.matmul(ps_b[:], dg[:],
                                     win(cc, b, tap, 13, 12, 26),
                                     start=(tap == 0), stop=(tap == 24))
                nc.scalar.activation(ftwin(ft, 0, 13), ps_a[:], AF.Copy)
                nc.scalar.activation(ftwin(ft, 13, 12), ps_b[:], AF.Copy)
            feat.append(ft)

        # head1: 1x1 conv + BN + ReLU (row-aligned splits over the 25x26 window)
        xr = []
        for mc in range(KC):
            t = xrp.tile([P, 626], F32R, tag="xr", name=f"xr{b}_{mc}")
            nc.vector.memset(t[:, 625:626].bitcast(F32), 0.0)
            xr.append(t)
        for mc in range(KC):
            for r0, nr in ((0, 13), (13, 12)):
                ph = ps_hd.tile([P, nr, 26], F32, tag="pshd")
                for kc in range(KC):
                    lhsT = h1w_sb[kc][:, mc * P:(mc + 1) * P]
                    nc.tensor.matmul(ph[:], lhsT, ftwin(feat[kc], r0, nr),
                                     start=(kc == 0), stop=(kc == 1))
                nc.scalar.activation(xr[mc][:, r0 * 25:(r0 + nr) * 25],
                                     ph[:, :, 0:25], AF.Relu, bias=h1b_sb[mc][:])

        # head2: 1x1 conv + bias
        for o0, n, nv in ((0, 320, 320), (320, 306, 305)):
            po = ps_hd.tile([OUT, n], F32, tag="pshd")
            for kc in range(KC):
                nc.tensor.matmul(po[:], h2w_sb[kc][:],
                                 xr[kc][:, o0:o0 + n],
                                 start=(kc == 0), stop=(kc == 1))
            nc.scalar.activation(out_sb[:, b * 625 + o0:b * 625 + o0 + nv], po[:, 0:nv],
                                 AF.Identity, bias=h2b_sb[:])

    nc.sync.dma_start(d["out"].ap(), out_sb[:])


def _build_program():
    if "nc" in _prog_cache:
        return _prog_cache["nc"]
    nc = bacc.Bacc("TRN2", target_bir_lowering=False, debug=False,
                   num_devices=NCORES)
    d = {}
    d["s_in"] = nc.dram_tensor("s_in", [KC, P, BPC, 31, 32], F32R, kind="ExternalInput")
    d["k_in"] = nc.dram_tensor("k_in", [KC, P, BPC, 9, 9], F32R, kind="ExternalInput")
    d["csw"] = nc.dram_tensor("csw", [KC, P, 9, 2, P], F32R, kind="ExternalInput")
    d["ckw"] = nc.dram_tensor("ckw", [KC, P, 9, 2, P], F32R, kind="ExternalInput")
    d["cs_bias"] = nc.dram_tensor("cs_bias", [KC, P, 1], F32, kind="ExternalInput")
    d["ck_bias"] = nc.dram_tensor("ck_bias", [KC, P, 1], F32, kind="ExternalInput")
    d["h1w"] = nc.dram_tensor("h1w", [KC, P, 2, P], F32R, kind="ExternalInput")
    d["h1_bias"] = nc.dram_tensor("h1_bias", [KC, P, 1], F32, kind="ExternalInput")
    d["h2w"] = nc.dram_tensor("h2w", [KC, P, OUT], F32R, kind="ExternalInput")
    d["h2_bias"] = nc.dram_tensor("h2_bias", [OUT, 1], F32, kind="ExternalInput")
    d["ident"] = nc.dram_tensor("ident", [P, P], F32, kind="ExternalInput")
    d["out"] = nc.dram_tensor("out", [OUT, BPC * 625], F32, kind="ExternalOutput")

    with tile.TileContext(nc) as tc:
        with ExitStack() as ctx:
            _emit(nc, tc, ctx, d)
    nc.compile()
    _prog_cache["nc"] = nc
    return nc


def kernel(**inputs):
    global LAST_RESULTS
    f32 = lambda x: np.ascontiguousarray(np.asarray(x), dtype=np.float32)
    kern, search = f32(inputs["kernel"]), f32(inputs["search"])

    # fold BN into conv weights / biases
    cks = f32(inputs["ck_g"]) / np.sqrt(f32(inputs["ck_v"]) + EPS)
    ckw_f = f32(inputs["ck_w"]) * cks[:, None, None, None]
    ckb = f32(inputs["ck_b"]) - f32(inputs["ck_m"]) * cks
    css = f32(inputs["cs_g"]) / np.sqrt(f32(inputs["cs_v"]) + EPS)
    csw_f = f32(inputs["cs_w"]) * css[:, None, None, None]
    csb = f32(inputs["cs_b"]) - f32(inputs["cs_m"]) * css
    h1s = f32(inputs["h_g"]) / np.sqrt(f32(inputs["h_v"]) + EPS)
    h1w_f = f32(inputs["h1_w"]) * h1s[:, None]
    h1b = f32(inputs["h_b"]) - f32(inputs["h_m"]) * h1s

    shared = {
        "csw": np.ascontiguousarray(csw_f.transpose(1, 2, 3, 0).reshape(KC, P, 9, 2, P)),
        "ckw": np.ascontiguousarray(ckw_f.transpose(1, 2, 3, 0).reshape(KC, P, 9, 2, P)),
        "cs_bias": csb.reshape(KC, P, 1),
        "ck_bias": ckb.reshape(KC, P, 1),
        "h1w": np.ascontiguousarray(h1w_f.transpose(1, 0).reshape(KC, P, 2, P)),
        "h1_bias": h1b.reshape(KC, P, 1),
        "h2w": np.ascontiguousarray(f32(inputs["h2_w"]).transpose(1, 0).reshape(KC, P, OUT)),
        "h2_bias": f32(inputs["h2_b"]).reshape(OUT, 1),
        "ident": np.eye(P, dtype=np.float32),
    }
    in_maps = []
    for i in range(NCORES):
        sl = slice(i * BPC, (i + 1) * BPC)
        m = dict(shared)
        s_pad = np.zeros((KC, P, BPC, 31, 32), np.float32)
        s_pad[..., :31] = search[sl].transpose(1, 0, 2, 3).reshape(KC, P, BPC, 31, 31)
        m["s_in"] = s_pad
        k_pad = np.zeros((KC, P, BPC, 9, 9), np.float32)
        k_pad[..., :7, :7] = kern[sl].transpose(1, 0, 2, 3).reshape(KC, P, BPC, 7, 7)
        m["k_in"] = k_pad
        in_maps.append(m)

    nc = _build_program()
    res = run_bass_kernel_spmd(nc, in_maps, core_ids=list(range(NCORES)))
    LAST_RESULTS = res
    out = np.empty((B, OUT, 25, 25), dtype=np.float32)
    for i in range(NCORES):
        o = res.results[i]["out"].reshape(OUT, BPC, 25, 25)
        out[i * BPC:(i + 1) * BPC] = o.transpose(1, 0, 2, 3)
    return out


# revision 15
# speedup vs baseline: 2.1787x; 1.2726x over previous
"""Trainium2 Bass kernel for DepthwiseXCorr (SiamRPN-style) model.

Pipeline (per sample): conv3x3+BN+ReLU on kernel & search branches,
depthwise cross-correlation, 1x1 conv + BN + ReLU head, 1x1 conv + bias.

Sharding: data-parallel over batch across 8 NeuronCores (8 samples each),
weights replicated.  BN is folded into conv weights on the host.

Layout on device: channels on SBUF partitions (2 chunks of 128), spatial x
batch on the free dimension.  Convolutions run as 9 shifted matmuls (fp32r
for full PE rate), the depthwise xcorr is split across PE (diagonal-matmul
trick), DVE and GPSIMD per (sample, channel-chunk) pair, tuned so every
engine finishes around the same time.
"""

import sys

if "/opt/trn_rl_repo" not in sys.path:
    sys.path.insert(0, "/opt/trn_rl_repo")

from contextlib import ExitStack

import numpy as np

import concourse.bass as bass
import concourse.tile as tile
from concourse import bacc, mybir
from concourse.bass_utils import run_bass_kernel_spmd

EPS = 1e-5
NCORES = 8
B, C, HID, OUT = 64, 256, 256, 10
BPC = B // NCORES  # samples per core
P = 128
KC = C // P  # channel chunks (2)
F32 = mybir.dt.float32
F32R = mybir.dt.float32r
AF = mybir.ActivationFunctionType
OP = mybir.AluOpType

# xcorr engine per (b, cc) pair, index p = b*2 + cc
# 't' = TensorE diag-matmul, 'v' = VectorE, 'g' = GpSimd
XC_ENGINE = ["v", "t", "v", "t", "v", "t", "v", "t",
             "v", "t", "v", "t", "v", "t", "v", "t"]

LAST_RESULTS = None  # BassKernelResults of the most recent run (for profiling)

_prog_cache = {}


def _emit(nc, tc, ctx, d):
    """Emit the per-core program.  d maps dram tensor name -> handle."""
    wp = ctx.enter_context(tc.tile_pool(name="weights", bufs=1))
    srp = ctx.enter_context(tc.tile_pool(name="srelu", bufs=1))
    krp = ctx.enter_context(tc.tile_pool(name="krelu", bufs=1))
    kp = ctx.enter_context(tc.tile_pool(name="kern", bufs=1))
    sp = ctx.enter_context(tc.tile_pool(name="search", bufs=6))
    featp = ctx.enter_context(tc.tile_pool(name="feat", bufs=6))
    diagp = ctx.enter_context(tc.tile_pool(name="diag", bufs=4))
    xrp = ctx.enter_context(tc.tile_pool(name="xrelu", bufs=6))
    outp = ctx.enter_context(tc.tile_pool(name="outs", bufs=1))
    ps_conv = ctx.enter_context(tc.tile_pool(name="ps_conv", bufs=3, space="PSUM"))
    ps_x = ctx.enter_context(tc.tile_pool(name="ps_x", bufs=2, space="PSUM"))
    ps_hd = ctx.enter_context(tc.tile_pool(name="ps_hd", bufs=3, space="PSUM"))

    # ---- weights / constants into SBUF ----
    csw_sb, ckw_sb, h1w_sb, h2w_sb = [], [], [], []
    csb_sb, ckb_sb, h1b_sb = [], [], []
    for kc in range(KC):
        t = wp.tile([P, 9 * 2 * P], F32R, tag=f"csw{kc}")
        nc.sync.dma_start(t[:], d["csw"].ap()[kc])
        csw_sb.append(t)
        t = wp.tile([P, 9 * 2 * P], F32R, tag=f"ckw{kc}")
        nc.sync.dma_start(t[:], d["ckw"].ap()[kc])
        ckw_sb.append(t)
        t = wp.tile([P, 2 * P], F32R, tag=f"h1w{kc}")
        nc.sync.dma_start(t[:], d["h1w"].ap()[kc])
        h1w_sb.append(t)
        t = wp.tile([P, OUT], F32R, tag=f"h2w{kc}")
        nc.sync.dma_start(t[:], d["h2w"].ap()[kc])
        h2w_sb.append(t)
    for mc in range(KC):
        t = wp.tile([P, 1], F32, tag=f"csb{mc}")
        nc.sync.dma_start(t[:], d["cs_bias"].ap()[mc])
        csb_sb.append(t)
        t = wp.tile([P, 1], F32, tag=f"ckb{mc}")
        nc.sync.dma_start(t[:], d["ck_bias"].ap()[mc])
        ckb_sb.append(t)
        t = wp.tile([P, 1], F32, tag=f"h1b{mc}")
        nc.sync.dma_start(t[:], d["h1_bias"].ap()[mc])
        h1b_sb.append(t)
    h2b_sb = wp.tile([OUT, 1], F32, tag="h2b")
    nc.sync.dma_start(h2b_sb[:], d["h2_bias"].ap())
    ident_sb = wp.tile([P, P], F32, tag="ident")
    nc.sync.dma_start(ident_sb[:], d["ident"].ap())

    # ---- inputs + convolutions (rotating per-sample search tiles) ----
    krelu_sb = [krp.tile([P, BPC * 25], F32, tag=f"krelu{mc}", name=f"krelu{mc}") for mc in range(KC)]
    srelu_sb = [srp.tile([P, BPC, 29, 30], F32R, tag=f"srelu{mc}", name=f"srelu{mc}") for mc in range(KC)]
    k_sb = []
    for kc in range(KC):
        t = kp.tile([P, BPC, 9, 9], F32R, tag=f"kin{kc}")
        nc.sync.dma_start(t[:], d["k_in"].ap()[kc])
        k_sb.append(t)

    # conv_kernel (3x3, BN+ReLU folded): k_relu[mc] = [128, b*25+tap]
    for mc in range(KC):
        psk = ps_conv.tile([P, BPC, 6, 6], F32, tag="pss")
        i = 0
        for tap in range(9):
            dy, dx = tap // 3, tap % 3
            for kc in range(KC):
                lhsT = ckw_sb[kc][:, tap * 2 * P + mc * P:tap * 2 * P + (mc + 1) * P]
                rhs = k_sb[kc][:, :, dy:dy + 6, dx:dx + 6]
                nc.tensor.matmul(psk[:], lhsT, rhs, start=(i == 0), stop=(i == 17))
                i += 1
        nc.scalar.activation(krelu_sb[mc][:], psk[:, :, 0:5, 0:5], AF.Relu, bias=ckb_sb[mc][:])

    # ---- depthwise xcorr + head, pipelined per sample ----
    out_sb = outp.tile([OUT, BPC * 625], F32, tag="osb")

    def kscalar(cc, b, tap):
        return krelu_sb[cc][:, b * 25 + tap:b * 25 + tap + 1]

    def win(cc, b, tap, r0=0, nr=25, w=25):
        dy, dx = tap // 5, tap % 5
        return srelu_sb[cc][:, b, dy + r0:dy + r0 + nr, dx:dx + w]

    def win746(cc, b, tap):
        # contiguous 746-elem span of the (dy,dx)-shifted window
        dy, dx = tap // 5, tap % 5
        flat = srelu_sb[cc][:].rearrange("p b y x -> p (b y x)")
        off = b * 870 + dy * 30 + dx
        return flat[:, off:off + 746]

    def ftwin(ft, r0, nr):
        # [nr, 26] row window of the 30-stride feat tile
        return ft[:].rearrange("p (y x) -> p y x", x=30)[:, r0:r0 + nr, 0:26]

    for b in range(BPC):
        # conv_search(b): s_relu[mc][b] = [29, 30] (col 29 = padding junk)
        s_sb = []
        for kc in range(KC):
            t = sp.tile([P, 31, 32], F32R, tag="sin", name=f"sin{kc}_{b}")
            nc.sync.dma_start(t[:], d["s_in"].ap()[kc, :, b])
            s_sb.append(t)
        for mc in range(KC):
            for y0, nr in ((0, 15), (15, 14)):
                pss = ps_conv.tile([P, nr, 30], F32, tag="pss")
                i = 0
                for tap in range(9):
                    dy, dx = tap // 3, tap % 3
                    for kc in range(KC):
                        lhsT = csw_sb[kc][:, tap * 2 * P + mc * P:tap * 2 * P + (mc + 1) * P]
                        rhs = s_sb[kc][:, y0 + dy:y0 + dy + nr, dx:dx + 30]
                        nc.tensor.matmul(pss[:], lhsT, rhs,
                                         start=(i == 0), stop=(i == 17))
                        i += 1
                nc.scalar.activation(srelu_sb[mc][:, b, y0:y0 + nr, :], pss[:],
                                     AF.Relu, bias=csb_sb[mc][:])

        # xcorr(b-1 pipelining handled by Tile deps)
        feat = []
        for cc in range(KC):
            eng = XC_ENGINE[b * 2 + cc]
            # ft holds the 25x25 xcorr output on a 30-element row stride
            # (cols 25..29 junk) so DVE ops can run one contiguous span.
            ft = featp.tile([P, 750], F32R, tag="feat")
            if eng == "v":
                e = nc.vector
                e.tensor_scalar(ft[:, 0:746], win746(cc, b, 0).bitcast(F32),
                                kscalar(cc, b, 0), None, OP.mult)
                for tap in range(1, 25):
                    e.scalar_tensor_tensor(ft[:, 0:746], win746(cc, b, tap).bitcast(F32),
                                           kscalar(cc, b, tap),
                                           ft[:, 0:746].bitcast(F32), OP.mult, OP.add)
            else:  # TensorE: accumulate diag(k_tap) @ shifted windows in PSUM
                ps_a = ps_x.tile([P, 13, 26], F32, tag="psx")
                ps_b = ps_x.tile([P, 12, 26], F32, tag="psx")
                for tap in range(25):
                    dg = diagp.tile([P, P], F32R, tag="diag")
                    nc.scalar.activation(dg[:], ident_sb[:], AF.Copy,
                                         scale=kscalar(cc, b, tap))
                    nc.tensor.matmul(ps_a[:], dg[:],
                                     win(cc, b, tap, 0, 13, 26),
                                     start=(tap == 0), stop=(tap == 24))
                    nc.tensor.matmul(ps_b[:], dg[:],
                                     win(cc, b, tap, 13, 12, 26),
                                     start=(tap == 0), stop=(tap == 24))
                nc.scalar.activation(ftwin(ft, 0, 13), ps_a[:], AF.Copy)
                nc.scalar.activation(ftwin(ft, 13, 12), ps_b[:], AF.Copy)
            feat.append(ft)

        # head1: 1x1 conv + BN + ReLU (row-aligned splits over the 25x26 window)
        xr = []
        for mc in range(KC):
            t = xrp.tile([P, 626], F32R, tag="xr", name=f"xr{b}_{mc}")
            nc.vector.memset(t[:, 625:626].bitcast(F32), 0.0)
            xr.append(t)
        for mc in range(KC):
            for r0, nr in ((0, 13), (13, 12)):
                ph = ps_hd.tile([P, nr, 26], F32, tag="pshd")
                for kc in range(KC):
                    lhsT = h1w_sb[kc][:, mc * P:(mc + 1) * P]
                    nc.tensor.matmul(ph[:], lhsT, ftwin(feat[kc], r0, nr),
                                     start=(kc == 0), stop=(kc == 1))
                nc.scalar.activation(xr[mc][:, r0 * 25:(r0 + nr) * 25],
                                     ph[:, :, 0:25], AF.Relu, bias=h1b_sb[mc][:])

        # head2: 1x1 conv + bias
        for o0, n, nv in ((0, 320, 320), (320, 306, 305)):
            po = ps_hd.tile([OUT, n], F32, tag="pshd")
            for kc in range(KC):
                nc.tensor.matmul(po[:], h2w_sb[kc][:],
                                 xr[kc][:, o0:o0 + n],
                                 start=(kc == 0), stop=(kc == 1))
            nc.scalar.activation(out_sb[:, b * 625 + o0:b * 625 + o0 + nv], po[:, 0:nv],
                                 AF.Identity, bias=h2b_sb[:])

    nc.sync.dma_start(d["out"].ap(), out_sb[:])


def _build_program():
    if "nc" in _prog_cache:
        return _prog_cache["nc"]
    nc = bacc.Bacc("TRN2", target_bir_lowering=False, debug=False,
                   num_devices=NCORES)
    d = {}
    d["s_in"] = nc.dram_tensor("s_in", [KC, P, BPC, 31, 32], F32R, kind="ExternalInput")
    d["k_in"] = nc.dram_tensor("k_in", [KC, P, BPC, 9, 9], F32R, kind="ExternalInput")
    d["csw"] = nc.dram_tensor("csw", [KC, P, 9, 2, P], F32R, kind="ExternalInput")
    d["ckw"] = nc.dram_tensor("ckw", [KC, P, 9, 2, P], F32R, kind="ExternalInput")
    d["cs_bias"] = nc.dram_tensor("cs_bias", [KC, P, 1], F32, kind="ExternalInput")
    d["ck_bias"] = nc.dram_tensor("ck_bias", [KC, P, 1], F32, kind="ExternalInput")
    d["h1w"] = nc.dram_tensor("h1w", [KC, P, 2, P], F32R, kind="ExternalInput")
    d["h1_bias"] = nc.dram_tensor("h1_bias", [KC, P, 1], F32, kind="ExternalInput")
    d["h2w"] = nc.dram_tensor("h2w", [KC, P, OUT], F32R, kind="ExternalInput")
    d["h2_bias"] = nc.dram_tensor("h2_bias", [OUT, 1], F32, kind="ExternalInput")
    d["ident"] = nc.dram_tensor("ident", [P, P], F32, kind="ExternalInput")
    d["out"] = nc.dram_tensor("out", [OUT, BPC * 625], F32, kind="ExternalOutput")

    with tile.TileContext(nc) as tc:
        with ExitStack() as ctx:
            _emit(nc, tc, ctx, d)
    nc.compile()
    _prog_cache["nc"] = nc
    return nc


def kernel(**inputs):
    global LAST_RESULTS
    f32 = lambda x: np.ascontiguousarray(np.asarray(x), dtype=np.float32)
    kern, search = f32(inputs["kernel"]), f32(inputs["search"])

    # fold BN into conv weights / biases
    cks = f32(inputs["ck_g"]) / np.sqrt(f32(inputs["ck_v"]) + EPS)
    ckw_f = f32(inputs["ck_w"]) * cks[:, None, None, None]
    ckb = f32(inputs["ck_b"]) - f32(inputs["ck_m"]) * cks
    css = f32(inputs["cs_g"]) / np.sqrt(f32(inputs["cs_v"]) + EPS)
    csw_f = f32(inputs["cs_w"]) * css[:, None, None, None]
    csb = f32(inputs["cs_b"]) - f32(inputs["cs_m"]) * css
    h1s = f32(inputs["h_g"]) / np.sqrt(f32(inputs["h_v"]) + EPS)
    h1w_f = f32(inputs["h1_w"]) * h1s[:, None]
    h1b = f32(inputs["h_b"]) - f32(inputs["h_m"]) * h1s

    shared = {
        "csw": np.ascontiguousarray(csw_f.transpose(1, 2, 3, 0).reshape(KC, P, 9, 2, P)),
        "ckw": np.ascontiguousarray(ckw_f.transpose(1, 2, 3, 0).reshape(KC, P, 9, 2, P)),
        "cs_bias": csb.reshape(KC, P, 1),
        "ck_bias": ckb.reshape(KC, P, 1),
        "h1w": np.ascontiguousarray(h1w_f.transpose(1, 0).reshape(KC, P, 2, P)),
        "h1_bias": h1b.reshape(KC, P, 1),
        "h2w": np.ascontiguousarray(f32(inputs["h2_w"]).transpose(1, 0).reshape(KC, P, OUT)),
        "h2_bias": f32(inputs["h2_b"]).reshape(OUT, 1),
        "ident": np.eye(P, dtype=np.float32),
    }
    in_maps = []
    for i in range(NCORES):
        sl = slice(i * BPC, (i + 1) * BPC)
        m = dict(shared)
        s_pad = np.zeros((KC, P, BPC, 31, 32), np.float32)
        s_pad[..., :31] = search[sl].transpose(1, 0, 2, 3).reshape(KC, P, BPC, 31, 31)
        m["s_in"] = s_pad
        k_pad = np.zeros((KC, P, BPC, 9, 9), np.float32)
        k_pad[..., :7, :7] = kern[sl].transpose(1, 0, 2, 3).reshape(KC, P, BPC, 7, 7)
        m["k_in"] = k_pad
        in_maps.append(m)

    nc = _build_program()
    res = run_bass_kernel_spmd(nc, in_maps, core_ids=list(range(NCORES)))
    LAST_RESULTS = res
    out = np.empty((B, OUT, 25, 25), dtype=np.float32)
    for i in range(NCORES):
        o = res.results[i]["out"].reshape(OUT, BPC, 25, 25)
        out[i * BPC:(i + 1) * BPC] = o.transpose(1, 0, 2, 3)
    return out


# revision 17
# speedup vs baseline: 2.2805x; 1.0467x over previous
"""Trainium2 Bass kernel for DepthwiseXCorr (SiamRPN-style) model.

Pipeline (per sample): conv3x3+BN+ReLU on kernel & search branches,
depthwise cross-correlation, 1x1 conv + BN + ReLU head, 1x1 conv + bias.

Sharding: data-parallel over batch across 8 NeuronCores (8 samples each),
weights replicated.  BN is folded into conv weights on the host.

Layout on device: channels on SBUF partitions (2 chunks of 128), spatial x
batch on the free dimension.  Convolutions run as 9 shifted matmuls (fp32r
for full PE rate), the depthwise xcorr is split across PE (diagonal-matmul
trick), DVE and GPSIMD per (sample, channel-chunk) pair, tuned so every
engine finishes around the same time.
"""

import sys

if "/opt/trn_rl_repo" not in sys.path:
    sys.path.insert(0, "/opt/trn_rl_repo")

from contextlib import ExitStack

import ml_dtypes
import numpy as np

import concourse.bass as bass
import concourse.tile as tile
from concourse import bacc, mybir
from concourse.bass_utils import run_bass_kernel_spmd

EPS = 1e-5
NCORES = 8
B, C, HID, OUT = 64, 256, 256, 10
BPC = B // NCORES  # samples per core
P = 128
KC = C // P  # channel chunks (2)
F32 = mybir.dt.float32
F32R = mybir.dt.float32r
BF16 = mybir.dt.bfloat16
AF = mybir.ActivationFunctionType
OP = mybir.AluOpType

# xcorr engine per (b, cc) pair, index p = b*2 + cc
# 't' = TensorE diag-matmul, 'v' = VectorE, 'g' = GpSimd
XC_ENGINE = ["v", "t", "v", "t", "v", "t", "v", "t",
             "v", "t", "v", "t", "v", "t", "v", "t"]

LAST_RESULTS = None  # BassKernelResults of the most recent run (for profiling)

_prog_cache = {}


def _emit(nc, tc, ctx, d):
    """Emit the per-core program.  d maps dram tensor name -> handle."""
    wp = ctx.enter_context(tc.tile_pool(name="weights", bufs=1))
    srp = ctx.enter_context(tc.tile_pool(name="srelu", bufs=1))
    krp = ctx.enter_context(tc.tile_pool(name="krelu", bufs=1))
    kp = ctx.enter_context(tc.tile_pool(name="kern", bufs=1))
    sp = ctx.enter_context(tc.tile_pool(name="search", bufs=6))
    featp = ctx.enter_context(tc.tile_pool(name="feat", bufs=6))
    diagp = ctx.enter_context(tc.tile_pool(name="diag", bufs=4))
    xrp = ctx.enter_context(tc.tile_pool(name="xrelu", bufs=6))
    outp = ctx.enter_context(tc.tile_pool(name="outs", bufs=1))
    ps_conv = ctx.enter_context(tc.tile_pool(name="ps_conv", bufs=3, space="PSUM"))
    ps_x = ctx.enter_context(tc.tile_pool(name="ps_x", bufs=2, space="PSUM"))
    ps_hd = ctx.enter_context(tc.tile_pool(name="ps_hd", bufs=3, space="PSUM"))

    # ---- weights / constants into SBUF ----
    csw_sb, ckw_sb, h1w_sb, h2w_sb = [], [], [], []
    csb_sb, ckb_sb, h1b_sb = [], [], []
    for kc in range(KC):
        t = wp.tile([P, 9 * 2 * P], BF16, tag=f"csw{kc}")
        nc.sync.dma_start(t[:], d["csw"].ap()[kc])
        csw_sb.append(t)
        t = wp.tile([P, 9 * 2 * P], F32R, tag=f"ckw{kc}")
        nc.sync.dma_start(t[:], d["ckw"].ap()[kc])
        ckw_sb.append(t)
        t = wp.tile([P, 2 * P], F32R, tag=f"h1w{kc}")
        nc.sync.dma_start(t[:], d["h1w"].ap()[kc])
        h1w_sb.append(t)
        t = wp.tile([P, OUT], F32R, tag=f"h2w{kc}")
        nc.sync.dma_start(t[:], d["h2w"].ap()[kc])
        h2w_sb.append(t)
    for mc in range(KC):
        t = wp.tile([P, 1], F32, tag=f"csb{mc}")
        nc.sync.dma_start(t[:], d["cs_bias"].ap()[mc])
        csb_sb.append(t)
        t = wp.tile([P, 1], F32, tag=f"ckb{mc}")
        nc.sync.dma_start(t[:], d["ck_bias"].ap()[mc])
        ckb_sb.append(t)
        t = wp.tile([P, 1], F32, tag=f"h1b{mc}")
        nc.sync.dma_start(t[:], d["h1_bias"].ap()[mc])
        h1b_sb.append(t)
    h2b_sb = wp.tile([OUT, 1], F32, tag="h2b")
    nc.sync.dma_start(h2b_sb[:], d["h2_bias"].ap())
    ident_sb = wp.tile([P, P], F32, tag="ident")
    nc.sync.dma_start(ident_sb[:], d["ident"].ap())

    # ---- inputs + convolutions (rotating per-sample search tiles) ----
    krelu_sb = [krp.tile([P, BPC * 25], F32, tag=f"krelu{mc}", name=f"krelu{mc}") for mc in range(KC)]
    srelu_sb = [srp.tile([P, BPC, 29, 30], BF16, tag=f"srelu{mc}", name=f"srelu{mc}") for mc in range(KC)]
    k_sb = []
    for kc in range(KC):
        t = kp.tile([P, BPC, 9, 9], F32R, tag=f"kin{kc}")
        nc.sync.dma_start(t[:], d["k_in"].ap()[kc])
        k_sb.append(t)

    # conv_kernel (3x3, BN+ReLU folded): k_relu[mc] = [128, b*25+tap]
    for mc in range(KC):
        psk = ps_conv.tile([P, BPC, 6, 6], F32, tag="pss")
        i = 0
        for tap in range(9):
            dy, dx = tap // 3, tap % 3
            for kc in range(KC):
                lhsT = ckw_sb[kc][:, tap * 2 * P + mc * P:tap * 2 * P + (mc + 1) * P]
                rhs = k_sb[kc][:, :, dy:dy + 6, dx:dx + 6]
                nc.tensor.matmul(psk[:], lhsT, rhs, start=(i == 0), stop=(i == 17))
                i += 1
        nc.scalar.activation(krelu_sb[mc][:], psk[:, :, 0:5, 0:5], AF.Relu, bias=ckb_sb[mc][:])

    # ---- depthwise xcorr + head, pipelined per sample ----
    out_sb = outp.tile([OUT, BPC * 625], F32, tag="osb")

    def kscalar(cc, b, tap):
        return krelu_sb[cc][:, b * 25 + tap:b * 25 + tap + 1]

    def win(cc, b, tap, r0=0, nr=25, w=25):
        dy, dx = tap // 5, tap % 5
        return srelu_sb[cc][:, b, dy + r0:dy + r0 + nr, dx:dx + w]

    def win746(cc, b, tap):
        # contiguous 746-elem span of the (dy,dx)-shifted window
        dy, dx = tap // 5, tap % 5
        flat = srelu_sb[cc][:].rearrange("p b y x -> p (b y x)")
        off = b * 870 + dy * 30 + dx
        return flat[:, off:off + 746]

    def ftwin(ft, r0, nr):
        # [nr, 26] row window of the 30-stride feat tile
        return ft[:].rearrange("p (y x) -> p y x", x=30)[:, r0:r0 + nr, 0:26]

    for b in range(BPC):
        # conv_search(b): s_relu[mc][b] = [29, 30] (col 29 = padding junk)
        s_sb = []
        for kc in range(KC):
            t = sp.tile([P, 31, 32], BF16, tag="sin", name=f"sin{kc}_{b}")
            nc.sync.dma_start(t[:], d["s_in"].ap()[kc, :, b])
            s_sb.append(t)
        for mc in range(KC):
            for y0, nr in ((0, 15), (15, 14)):
                pss = ps_conv.tile([P, nr, 30], F32, tag="pss")
                i = 0
                for tap in range(9):
                    dy, dx = tap // 3, tap % 3
                    for kc in range(KC):
                        lhsT = csw_sb[kc][:, tap * 2 * P + mc * P:tap * 2 * P + (mc + 1) * P]
                        rhs = s_sb[kc][:, y0 + dy:y0 + dy + nr, dx:dx + 30]
                        nc.tensor.matmul(pss[:], lhsT, rhs,
                                         start=(i == 0), stop=(i == 17))
                        i += 1
                nc.scalar.activation(srelu_sb[mc][:, b, y0:y0 + nr, :], pss[:],
                                     AF.Relu, bias=csb_sb[mc][:])

        # xcorr(b-1 pipelining handled by Tile deps)
        feat = []
        for cc in range(KC):
            eng = XC_ENGINE[b * 2 + cc]
            # ft holds the 25x25 xcorr output on a 30-element row stride
            # (cols 25..29 junk) so DVE ops can run one contiguous span.
            ft = featp.tile([P, 750], F32R, tag="feat")
            if eng == "v":
                e = nc.vector
                e.tensor_scalar(ft[:, 0:746], win746(cc, b, 0),
                                kscalar(cc, b, 0), None, OP.mult)
                for tap in range(1, 25):
                    e.scalar_tensor_tensor(ft[:, 0:746], win746(cc, b, tap),
                                           kscalar(cc, b, tap),
                                           ft[:, 0:746].bitcast(F32), OP.mult, OP.add)
            else:  # TensorE: accumulate diag(k_tap) @ shifted windows in PSUM
                ps_a = ps_x.tile([P, 13, 26], F32, tag="psx")
                ps_b = ps_x.tile([P, 12, 26], F32, tag="psx")
                for tap in range(25):
                    dg = diagp.tile([P, P], BF16, tag="diag")
                    nc.scalar.activation(dg[:], ident_sb[:], AF.Copy,
                                         scale=kscalar(cc, b, tap))
                    nc.tensor.matmul(ps_a[:], dg[:],
                                     win(cc, b, tap, 0, 13, 26),
                                     start=(tap == 0), stop=(tap == 24))
                    nc.tensor.matmul(ps_b[:], dg[:],
                                     win(cc, b, tap, 13, 12, 26),
                                     start=(tap == 0), stop=(tap == 24))
                nc.scalar.activation(ftwin(ft, 0, 13), ps_a[:], AF.Copy)
                nc.scalar.activation(ftwin(ft, 13, 12), ps_b[:], AF.Copy)
            feat.append(ft)

        # head1: 1x1 conv + BN + ReLU (row-aligned splits over the 25x26 window)
        xr = []
        for mc in range(KC):
            t = xrp.tile([P, 626], F32R, tag="xr", name=f"xr{b}_{mc}")
            nc.vector.memset(t[:, 625:626].bitcast(F32), 0.0)
            xr.append(t)
        for mc in range(KC):
            for r0, nr in ((0, 13), (13, 12)):
                ph = ps_hd.tile([P, nr, 26], F32, tag="pshd")
                for kc in range(KC):
                    lhsT = h1w_sb[kc][:, mc * P:(mc + 1) * P]
                    nc.tensor.matmul(ph[:], lhsT, ftwin(feat[kc], r0, nr),
                                     start=(kc == 0), stop=(kc == 1))
                nc.scalar.activation(xr[mc][:, r0 * 25:(r0 + nr) * 25],
                                     ph[:, :, 0:25], AF.Relu, bias=h1b_sb[mc][:])

        # head2: 1x1 conv + bias
        for o0, n, nv in ((0, 320, 320), (320, 306, 305)):
            po = ps_hd.tile([OUT, n], F32, tag="pshd")
            for kc in range(KC):
                nc.tensor.matmul(po[:], h2w_sb[kc][:],
                                 xr[kc][:, o0:o0 + n],
                                 start=(kc == 0), stop=(kc == 1))
            nc.scalar.activation(out_sb[:, b * 625 + o0:b * 625 + o0 + nv], po[:, 0:nv],
                                 AF.Identity, bias=h2b_sb[:])

    nc.sync.dma_start(d["out"].ap(), out_sb[:])


def _build_program():
    if "nc" in _prog_cache:
        return _prog_cache["nc"]
    nc = bacc.Bacc("TRN2", target_bir_lowering=False, debug=False,
                   num_devices=NCORES)
    d = {}
    d["s_in"] = nc.dram_tensor("s_in", [KC, P, BPC, 31, 32], BF16, kind="ExternalInput")
    d["k_in"] = nc.dram_tensor("k_in", [KC, P, BPC, 9, 9], F32R, kind="ExternalInput")
    d["csw"] = nc.dram_tensor("csw", [KC, P, 9, 2, P], BF16, kind="ExternalInput")
    d["ckw"] = nc.dram_tensor("ckw", [KC, P, 9, 2, P], F32R, kind="ExternalInput")
    d["cs_bias"] = nc.dram_tensor("cs_bias", [KC, P, 1], F32, kind="ExternalInput")
    d["ck_bias"] = nc.dram_tensor("ck_bias", [KC, P, 1], F32, kind="ExternalInput")
    d["h1w"] = nc.dram_tensor("h1w", [KC, P, 2, P], F32R, kind="ExternalInput")
    d["h1_bias"] = nc.dram_tensor("h1_bias", [KC, P, 1], F32, kind="ExternalInput")
    d["h2w"] = nc.dram_tensor("h2w", [KC, P, OUT], F32R, kind="ExternalInput")
    d["h2_bias"] = nc.dram_tensor("h2_bias", [OUT, 1], F32, kind="ExternalInput")
    d["ident"] = nc.dram_tensor("ident", [P, P], F32, kind="ExternalInput")
    d["out"] = nc.dram_tensor("out", [OUT, BPC * 625], F32, kind="ExternalOutput")

    with tile.TileContext(nc) as tc:
        with ExitStack() as ctx:
            _emit(nc, tc, ctx, d)
    nc.compile()
    _prog_cache["nc"] = nc
    return nc


def kernel(**inputs):
    global LAST_RESULTS
    f32 = lambda x: np.ascontiguousarray(np.asarray(x), dtype=np.float32)
    kern, search = f32(inputs["kernel"]), f32(inputs["search"])

    # fold BN into conv weights / biases
    cks = f32(inputs["ck_g"]) / np.sqrt(f32(inputs["ck_v"]) + EPS)
    ckw_f = f32(inputs["ck_w"]) * cks[:, None, None, None]
    ckb = f32(inputs["ck_b"]) - f32(inputs["ck_m"]) * cks
    css = f32(inputs["cs_g"]) / np.sqrt(f32(inputs["cs_v"]) + EPS)
    csw_f = f32(inputs["cs_w"]) * css[:, None, None, None]
    csb = f32(inputs["cs_b"]) - f32(inputs["cs_m"]) * css
    h1s = f32(inputs["h_g"]) / np.sqrt(f32(inputs["h_v"]) + EPS)
    h1w_f = f32(inputs["h1_w"]) * h1s[:, None]
    h1b = f32(inputs["h_b"]) - f32(inputs["h_m"]) * h1s

    shared = {
        "csw": np.ascontiguousarray(
            csw_f.transpose(1, 2, 3, 0).reshape(KC, P, 9, 2, P)).astype(ml_dtypes.bfloat16),
        "ckw": np.ascontiguousarray(ckw_f.transpose(1, 2, 3, 0).reshape(KC, P, 9, 2, P)),
        "cs_bias": csb.reshape(KC, P, 1),
        "ck_bias": ckb.reshape(KC, P, 1),
        "h1w": np.ascontiguousarray(h1w_f.transpose(1, 0).reshape(KC, P, 2, P)),
        "h1_bias": h1b.reshape(KC, P, 1),
        "h2w": np.ascontiguousarray(f32(inputs["h2_w"]).transpose(1, 0).reshape(KC, P, OUT)),
        "h2_bias": f32(inputs["h2_b"]).reshape(OUT, 1),
        "ident": np.eye(P, dtype=np.float32),
    }
    in_maps = []
    for i in range(NCORES):
        sl = slice(i * BPC, (i + 1) * BPC)
        m = dict(shared)
        s_pad = np.zeros((KC, P, BPC, 31, 32), ml_dtypes.bfloat16)
        s_pad[..., :31] = search[sl].transpose(1, 0, 2, 3).reshape(KC, P, BPC, 31, 31)
        m["s_in"] = s_pad
        k_pad = np.zeros((KC, P, BPC, 9, 9), np.float32)
        k_pad[..., :7, :7] = kern[sl].transpose(1, 0, 2, 3).reshape(KC, P, BPC, 7, 7)
        m["k_in"] = k_pad
        in_maps.append(m)

    nc = _build_program()
    res = run_bass_kernel_spmd(nc, in_maps, core_ids=list(range(NCORES)))
    LAST_RESULTS = res
    out = np.empty((B, OUT, 25, 25), dtype=np.float32)
    for i in range(NCORES):
        o = res.results[i]["out"].reshape(OUT, BPC, 25, 25)
        out[i * BPC:(i + 1) * BPC] = o.transpose(1, 0, 2, 3)
    return out
